# revision 9
# baseline (speedup 1.0000x reference)
"""Trainium2 Bass kernel for nn_GroundingNet (gnn_message_passing).

Data-parallel over batch: 32 batches -> 8 cores x 4 batches.
Node path: x[2048,1024] MLP (1024-512-256-128-8), computed transposed
  (features on partitions) so weights load as natural lhsT and BN+ReLU /
  sigmoid fuse into the PSUM->SBUF eviction on the scalar engine.
Edge path: host gathers per-edge node attrs into a quarter-stacked
  [128, E/4] tensor (4 edge-quarters x 32 attr rows); device runs the
  18-64-64-10 MLP as packed/block-diagonal matmuls.
"""

import re
import time

import ml_dtypes
import numpy as np

import bass_rust
import concourse.bass as bass
import concourse.mybir as mybir
from concourse import tile
from concourse.bass_utils import run_bass_kernel_spmd
from concourse.tile import TileContext

BF16 = ml_dtypes.bfloat16
F32 = mybir.dt.float32
BF = mybir.dt.bfloat16

B, N, F = 32, 512, 1024
E = 32768
H1, H2, H3, NCC = 512, 256, 128, 8
EH, ECC = 64, 10
NCORES = 8
BPC = B // NCORES          # 4 batches per core
R = BPC * N                # 2048 node rows per core
Q = E // 4                 # 8192 edges per quarter
BBOX_POS_MAX = 1024.0
BN_EPS = 1e-5

AF = mybir.ActivationFunctionType
last_spmd_wall_ns = None


# ---------------------------------------------------------------- tile patch
# walrus in this container allows only ONE sem wait on a drain (CTRL) op;
# split the TileContext exit drain into one drain per ticked proc.
def _split_drain_and_barrier(self, tick_clock, wait_clock):
    gc = tick_clock.global_clock
    ticks = [int(s) for s in re.findall(r"\d+", repr(gc))]
    nz = [i for i, t in enumerate(ticks) if t > 0]
    chunks = [[p] for p in nz] or [[]]
    for chunk in chunks:
        vec = [0] * len(ticks)
        for p in chunk:
            vec[p] = ticks[p]
        d = self.nc.sync.drain()
        wait_clock.add_sem_waits(
            d.ins, bass_rust.ScopedClock({None: bass_rust.VectorClock(vec)})
        )
    self.nc.all_engine_barrier()
    popped = self.nc._tile_sem_poison_stack.pop()
    assert popped is self._sem_poison
    self.nc.clear_and_free_semaphores(list(self.sems.allocated().values()))
    self.nc.all_engine_barrier()


TileContext._drain_and_barrier = _split_drain_and_barrier

# The same walrus rejects >2 sem waits on any instruction (and >1 on CTRL
# types). Before Tile lowers the scheduled instruction lists into basic
# blocks, hoist all-but-one sem waits onto no-fuse carrier nops (one wait
# each) inserted just before the offending instruction.
_orig_lower = TileContext._lower_ordered_insts


def _split_lower(self, ordered):
    for bb_name, insts in ordered.items():
        new_list = []
        for inst in insts:
            si = inst.sync_info
            waits = list(si.on_wait) if si is not None else []
            lim = 1
            if len(waits) > lim:
                extra, keep = waits[:-lim], waits[-lim:]
                for w in extra:
                    nop = self.nc.engines[inst.engine].nop(nofuse=True).ins
                    nop.sync_info = bass_rust.SyncInfo(
                        on_wait=[w], on_update=[]
                    )
                    new_list.append(nop)
                inst.sync_info = bass_rust.SyncInfo(
                    on_wait=keep, on_update=list(si.on_update)
                )
            new_list.append(inst)
        ordered[bb_name] = new_list
    return _orig_lower(self, ordered)


TileContext._lower_ordered_insts = _split_lower


# ---------------------------------------------------------------- device code
def build_nc():
    nc = bass.Bass()
    dp = nc.declare_dram_parameter

    x_d = dp("xT", [8, 128, R], BF, isOutput=False)
    w1_d = dp("w1", [8, 4, 128, 128], BF, isOutput=False)
    w2_d = dp("w2", [4, 2, 128, 128], BF, isOutput=False)
    w3_d = dp("w3", [2, 128, 128], BF, isOutput=False)
    w4_d = dp("w4", [128, NCC], BF, isOutput=False)
    s1_d = dp("s1", [128, 4], F32, isOutput=False)
    b1_d = dp("b1", [128, 4], F32, isOutput=False)
    s2_d = dp("s2", [128, 2], F32, isOutput=False)
    b2_d = dp("b2", [128, 2], F32, isOutput=False)
    b3_d = dp("b3", [128, 1], F32, isOutput=False)
    b4_d = dp("b4", [NCC, 1], F32, isOutput=False)

    attr_d = dp("attr", [BPC, 128, Q], BF, isOutput=False)
    w1s_d = dp("w1s", [128, 64], BF, isOutput=False)
    w2bd_d = dp("w2bd", [128, 128], BF, isOutput=False)
    w3bd_d = dp("w3bd", [128, 4, 80], BF, isOutput=False)
    b1e_d = dp("b1e", [128, 1], F32, isOutput=False)
    b2e_d = dp("b2e", [128, 1], F32, isOutput=False)
    b3e_d = dp("b3e", [80, 1], F32, isOutput=False)

    nout_d = dp("nodeT", [NCC, R], F32, isOutput=True)
    eout_d = dp("edgeT", [BPC, ECC, E], F32, isOutput=True)

    with TileContext(nc) as tc:
        with (
            tc.tile_pool(name="wpool", bufs=1) as wp,
            tc.tile_pool(name="xpool", bufs=1) as xp,
            tc.tile_pool(name="hpool", bufs=1) as hp,
            tc.tile_pool(name="apool", bufs=2) as ap,
            tc.tile_pool(name="epool", bufs=3) as ep,
            tc.tile_pool(name="opool", bufs=3) as op,
            tc.tile_pool(name="npsum", bufs=2, space="PSUM") as nps,
            tc.tile_pool(name="e12psum", bufs=1, space="PSUM") as eps12,
            tc.tile_pool(name="e3psum", bufs=2, space="PSUM") as eps3,
        ):
            # ---------------- resident weights / params
            w1_t = wp.tile([128, 8, 4, 128], BF)
            w2_t = wp.tile([128, 4, 2, 128], BF)
            w3_t = wp.tile([128, 2, 128], BF)
            w4_t = wp.tile([128, NCC], BF)
            for k in range(8):
                nc.sync.dma_start(w1_t[:, k], w1_d[k])
            for k in range(4):
                nc.sync.dma_start(w2_t[:, k], w2_d[k])
            nc.sync.dma_start(w3_t[:], w3_d[:])
            nc.sync.dma_start(w4_t[:], w4_d[:])
            s1_t = wp.tile([128, 4], F32)
            b1_t = wp.tile([128, 4], F32)
            s2_t = wp.tile([128, 2], F32)
            b2_t = wp.tile([128, 2], F32)
            b3_t = wp.tile([128, 1], F32)
            b4_t = wp.tile([NCC, 1], F32)
            for t, d in [(s1_t, s1_d), (b1_t, b1_d), (s2_t, s2_d),
                         (b2_t, b2_d), (b3_t, b3_d), (b4_t, b4_d)]:
                nc.sync.dma_start(t[:], d[:])
            w1s_t = wp.tile([128, 64], BF)
            w2bd_t = wp.tile([128, 128], BF)
            w3bd_t = wp.tile([128, 4, 80], BF)
            b1e_t = wp.tile([128, 1], F32)
            b2e_t = wp.tile([128, 1], F32)
            b3e_t = wp.tile([80, 1], F32)
            for t, d in [(w1s_t, w1s_d), (w2bd_t, w2bd_d), (w3bd_t, w3bd_d),
                         (b1e_t, b1e_d), (b2e_t, b2e_d), (b3e_t, b3e_d)]:
                nc.sync.dma_start(t[:], d[:])

            # ---------------- node path (transposed: features on partitions)
            x_t = xp.tile([128, 8, R], BF)
            for k in range(8):
                nc.sync.dma_start(x_t[:, k], x_d[k])
            h1_t = hp.tile([128, 4, R], BF)
            h2_t = hp.tile([128, 2, R], BF)
            h3_t = hp.tile([128, R], BF)
            no_t = hp.tile([NCC, R], F32)

            for m in range(4):          # H1 chunks
                for n in range(4):      # row tiles of 512
                    ps = nps.tile([128, 512], F32)
                    for k in range(8):
                        nc.tensor.matmul(
                            ps[:], w1_t[:, k, m], x_t[:, k, bass.ts(n, 512)],
                            start=(k == 0), stop=(k == 7),
                        )
                    if (m * 4 + n) % 2 == 0:
                        nc.scalar.activation(
                            h1_t[:, m, bass.ts(n, 512)], ps[:], AF.Relu,
                            bias=b1_t[:, m : m + 1],
                        )
                    else:
                        nc.vector.tensor_scalar(
                            h1_t[:, m, bass.ts(n, 512)], ps[:],
                            b1_t[:, m : m + 1], 0.0,
                            mybir.AluOpType.add, mybir.AluOpType.max,
                        )
            for m in range(2):          # H2 chunks
                for n in range(4):
                    ps = nps.tile([128, 512], F32)
                    for k in range(4):
                        nc.tensor.matmul(
                            ps[:], w2_t[:, k, m], h1_t[:, k, bass.ts(n, 512)],
                            start=(k == 0), stop=(k == 3),
                        )
                    if (m * 4 + n) % 2 == 0:
                        nc.vector.tensor_scalar(
                            h2_t[:, m, bass.ts(n, 512)], ps[:],
                            b2_t[:, m : m + 1], 0.0,
                            mybir.AluOpType.add, mybir.AluOpType.max,
                        )
                    else:
                        nc.scalar.activation(
                            h2_t[:, m, bass.ts(n, 512)], ps[:], AF.Relu,
                            bias=b2_t[:, m : m + 1],
                        )
            for n in range(4):          # H3
                ps = nps.tile([128, 512], F32)
                for k in range(2):
                    nc.tensor.matmul(
                        ps[:], w3_t[:, k], h2_t[:, k, bass.ts(n, 512)],
                        start=(k == 0), stop=(k == 1),
                    )
                if n % 2 == 0:
                    nc.vector.tensor_scalar(
                        h3_t[:, bass.ts(n, 512)], ps[:], b3_t[:, 0:1], 0.0,
                        mybir.AluOpType.add, mybir.AluOpType.max,
                    )
                else:
                    nc.scalar.activation(
                        h3_t[:, bass.ts(n, 512)], ps[:], AF.Relu,
                        bias=b3_t[:, 0:1],
                    )
            for n in range(4):          # output concepts
                ps = nps.tile([128, 512], F32)
                nc.tensor.matmul(ps[0:NCC], w4_t[:], h3_t[:, bass.ts(n, 512)],
                                 start=True, stop=True)
                nc.scalar.activation(
                    no_t[:, bass.ts(n, 512)], ps[0:NCC], AF.Sigmoid,
                    bias=b4_t[:, 0:1],
                )
            nc.sync.dma_start(nout_d[:], no_t[:])

            # ---------------- edge path
            for b in range(BPC):
                attr_t = ap.tile([128, Q], BF)
                nc.sync.dma_start(attr_t[:], attr_d[b])
                h2buf = []          # (tile, col offset j0) pending for L3
                for r in range(Q // 512):   # 16 rounds of 512 cols
                    j0 = r * 512
                    ps1 = eps12.tile([128, 1024], F32, name="ps1")
                    # L1: 4 packed matmuls K=32 M=64 (quarters on row strips)
                    for g in range(4):
                        out = ps1[64 * (g % 2) : 64 * (g % 2) + 64,
                                  512 * (g // 2) : 512 * (g // 2) + 512]
                        nc.tensor.matmul(
                            out,
                            w1s_t[32 * g : 32 * g + 32],
                            attr_t[32 * g : 32 * g + 32, j0 : j0 + 512],
                            start=True, stop=True,
                            tile_position=(32 * g, 64 * (g % 2)),
                        )
                    h1e = ep.tile([128, 1024], BF, name="h1e")
                    eng = nc.scalar if r % 2 == 0 else nc.vector
                    if r % 2 == 0:
                        nc.scalar.activation(h1e[:], ps1[:], AF.Relu,
                                             bias=b1e_t[:, 0:1])
                    else:
                        nc.vector.tensor_scalar(
                            h1e[:], ps1[:], b1e_t[:, 0:1], 0.0,
                            mybir.AluOpType.add, mybir.AluOpType.max,
                        )
                    # L2: block-diag, one MM per 512-col half
                    ps2 = eps12.tile([128, 1024], F32, name="ps2")
                    for half in range(2):
                        nc.tensor.matmul(
                            ps2[:, bass.ts(half, 512)], w2bd_t[:],
                            h1e[:, bass.ts(half, 512)],
                            start=True, stop=True,
                        )
                    h2e = ep.tile([128, 1024], BF, name="h2e")
                    if r % 2 == 0:
                        nc.vector.tensor_scalar(
                            h2e[:], ps2[:], b2e_t[:, 0:1], 0.0,
                            mybir.AluOpType.add, mybir.AluOpType.max,
                        )
                    else:
                        nc.scalar.activation(h2e[:], ps2[:], AF.Relu,
                                             bias=b2e_t[:, 0:1])
                    h2buf.append((h2e, j0))
                    if len(h2buf) == 2:
                        # L3: 4 accumulating block-diag chunks -> [80, 512]
                        ps3 = eps3.tile([128, 512], F32, name="ps3")
                        chunks = [(h2buf[0][0], 0), (h2buf[0][0], 1),
                                  (h2buf[1][0], 0), (h2buf[1][0], 1)]
                        for c, (t, half) in enumerate(chunks):
                            nc.tensor.matmul(
                                ps3[0:80], w3bd_t[:, c],
                                t[:, bass.ts(half, 512)],
                                start=(c == 0), stop=(c == 3),
                            )
                        ev = op.tile([80, 512], F32, name="ev")
                        nc.scalar.activation(ev[:], ps3[0:80], AF.Sigmoid,
                                             bias=b3e_t[:, 0:1])
                        # slabs: chunk c covers quarters (2*(c%2), 2*(c%2)+1)
                        # at col offset h2buf[c//2][1] + 512*(c%2)
                        for c in range(4):
                            jj = h2buf[c // 2][1]
                            # rows 20c+0..9 -> quarter 2*(c%2) (rhs parts 0-63),
                            # rows 20c+10..19 -> quarter 2*(c%2)+1 (parts 64-127)
                            qa = 2 * (c % 2)
                            nc.sync.dma_start(
                                eout_d[b][:, qa * Q + jj : qa * Q + jj + 512],
                                ev[20 * c : 20 * c + 10],
                            )
                            nc.sync.dma_start(
                                eout_d[b][:, (qa + 1) * Q + jj : (qa + 1) * Q + jj + 512],
                                ev[20 * c + 10 : 20 * c + 20],
                            )
                        h2buf = []
    return nc


# ---------------------------------------------------------------- host glue
def _host_prep(inputs):
    eps = BN_EPS
    f32 = np.float32
    w = {}
    # node weights
    a1f = (inputs["bn1_g"] / np.sqrt(inputs["bn1_rv"] + eps)).astype(np.float64)
    a2f = (inputs["bn2_g"] / np.sqrt(inputs["bn2_rv"] + eps)).astype(np.float64)
    w["w1"] = np.ascontiguousarray(
        (inputs["np_w1"] * a1f[None, :]).reshape(8, 128, 4, 128).transpose(0, 2, 1, 3)
    ).astype(BF16)
    w["w2"] = np.ascontiguousarray(
        (inputs["np_w2"] * a2f[None, :]).reshape(4, 128, 2, 128).transpose(0, 2, 1, 3)
    ).astype(BF16)
    w["w3"] = np.ascontiguousarray(inputs["np_w3"].reshape(2, 128, 128)).astype(BF16)
    w["w4"] = inputs["ni_w"].astype(BF16)
    a1 = (inputs["bn1_g"] / np.sqrt(inputs["bn1_rv"] + eps)).astype(f32)
    c1 = a1 * (inputs["np_b1"] - inputs["bn1_rm"]) + inputs["bn1_b"]
    a2 = (inputs["bn2_g"] / np.sqrt(inputs["bn2_rv"] + eps)).astype(f32)
    c2 = a2 * (inputs["np_b2"] - inputs["bn2_rm"]) + inputs["bn2_b"]
    w["s1"] = np.ones((128, 4), f32)
    w["b1"] = np.ascontiguousarray(c1.reshape(4, 128).T).astype(f32)
    w["s2"] = np.ones((128, 2), f32)
    w["b2"] = np.ascontiguousarray(c2.reshape(2, 128).T).astype(f32)
    w["b3"] = inputs["np_b3"].reshape(128, 1).astype(f32)
    w["b4"] = inputs["ni_b"].reshape(NCC, 1).astype(f32)
    # edge weights
    ew1, ew2, ei_w = inputs["ep_w1"], inputs["ep_w2"], inputs["ei_w"]
    w1pad = np.zeros((32, 64), f32)
    w1pad[0:9] = ew1[0:9]
    w1pad[16:25] = ew1[9:18]
    w["w1s"] = np.tile(w1pad, (4, 1)).astype(BF16)
    w2bd = np.zeros((128, 128), f32)
    w2bd[0:64, 0:64] = ew2
    w2bd[64:128, 64:128] = ew2
    w["w2bd"] = w2bd.astype(BF16)
    w3bd = np.zeros((128, 4, 80), f32)
    for c in range(4):
        w3bd[0:64, c, 20 * c : 20 * c + 10] = ei_w
        w3bd[64:128, c, 20 * c + 10 : 20 * c + 20] = ei_w
    w["w3bd"] = w3bd.astype(BF16)
    w["b1e"] = np.tile(inputs["ep_b1"], 2).reshape(128, 1).astype(f32)
    w["b2e"] = np.tile(inputs["ep_b2"], 2).reshape(128, 1).astype(f32)
    w["b3e"] = np.tile(inputs["ei_b"], 8).reshape(80, 1).astype(f32)
    return w


def kernel(**inputs):
    nc = build_nc()
    w = _host_prep(inputs)

    roi = inputs["roi_features"]
    attr9 = np.concatenate(
        [
            inputs["batch_bboxes"] / BBOX_POS_MAX,
            inputs["batch_directions"],
            inputs["batch_priorities"][..., None],
        ],
        axis=-1,
    ).astype(np.float32)                      # [B, N, 9]
    eidx = inputs["batch_edge_index"]         # [B, 2, E] int32

    in_maps = []
    for c in range(NCORES):
        bs = slice(BPC * c, BPC * (c + 1))
        xT = np.ascontiguousarray(
            roi[bs].reshape(R, F).T.reshape(8, 128, R)
        ).astype(BF16)
        attr = np.zeros((BPC, 4, 32, Q), np.float32)
        for bi, b in enumerate(range(BPC * c, BPC * (c + 1))):
            src = attr9[b][eidx[b, 0]].T      # [9, E]
            dst = attr9[b][eidx[b, 1]].T
            attr[bi, :, 0:9] = src.reshape(9, 4, Q).transpose(1, 0, 2)
            attr[bi, :, 16:25] = dst.reshape(9, 4, Q).transpose(1, 0, 2)
        m = {"xT": xT, "attr": attr.reshape(BPC, 128, Q).astype(BF16)}
        m.update(w)
        in_maps.append(m)

    global last_spmd_wall_ns
    t0 = time.monotonic_ns()
    res = run_bass_kernel_spmd(nc, in_maps, list(range(NCORES)))
    last_spmd_wall_ns = time.monotonic_ns() - t0

    node = np.empty((B, N, NCC), np.float32)
    edge = np.empty((B * E, ECC), np.float32)
    for c in range(NCORES):
        r = res.results[c]
        node[BPC * c : BPC * (c + 1)] = r["nodeT"].T.reshape(BPC, N, NCC)
        for bi in range(BPC):
            b = BPC * c + bi
            edge[b * E : (b + 1) * E] = r["edgeT"][bi].T
    return node, edge
